# revision 1
# baseline (speedup 1.0000x reference)
"""DomainAwareGAT (2-layer GATv2 + LN + ELU + residual) on 8 Trainium2 cores.

Strategy: shard edges by destination-node range (core k owns dst rows
[k*2500, (k+1)*2500)). Node features replicated; per layer the full
xl = x@Wl GEMM is computed replicated on every core (cheaper than an
allgather of xl), xr only for local rows. Edges are sorted by dst on the
host and processed in 128-node output blocks; per 128-edge chunk a one-hot
(edge -> local node) matrix M built on-chip turns segment-sum into PE
matmuls (den = M^T @ ex, U = M^T @ (ex * xl[src])), with the softmax
normalization applied per node (out = U/den) instead of per edge; the
softmax max-subtraction is dropped (shift-invariant, logits are O(1)).
The only cross-core communication is an AllGather of the residual state
between the two layers.
"""
import os
import sys

sys.path.insert(0, "/opt/trn_rl_repo")

import numpy as np
import ml_dtypes

import concourse.bass as bass
import concourse.tile as tile
from concourse import bacc, mybir
from concourse.bass_utils import run_bass_kernel_spmd

F32 = mybir.dt.float32
BF16 = mybir.dt.bfloat16
I16 = mybir.dt.int16
AF = mybir.ActivationFunctionType
ALU = mybir.AluOpType

N, E, D, H, C, L = 20000, 320000, 256, 8, 32, 2
NEG_SLOPE = 0.2
LN_EPS = 1e-5
NCORES = 8
NLOC = N // NCORES            # 2500 real nodes per core
PPC = 2560                    # padded nodes per core (20 x 128)
NPAD = NCORES * PPC           # 20480-row padded node space (160 x 128)
NBLK = (NLOC + 127) // 128    # 20 output blocks per core (last = 68 rows)
P = 128


# ---------------------------------------------------------------- host prep
def _pack_idxs(e_list):
    """Pack a flat gather-index list into dma_gather's [128, n/16] layout:
    arr[a, c*8+g] = e_list[c*128 + a + 16*g], replicated over 8 Q7 cores,
    so that out[p, c, :] = table[e_list[c*128 + p]]."""
    nch = len(e_list) // P
    e3 = np.asarray(e_list, np.int16).reshape(nch, 8, 16)  # [c, g, a]
    return np.tile(e3.transpose(2, 0, 1).reshape(16, nch * 8), (8, 1))


def _col_layout(arr):
    """[totch*128] edge-order array -> [128, totch] (chunk c in column c)."""
    return np.ascontiguousarray(arr.reshape(-1, P).T)


def _prep_edges(edge_index, edge_attr):
    """Bucket edges by dst core, sort by dst, pad blocks to common chunk
    counts shared by all cores (SPMD: one program, same loop bounds)."""
    src = np.asarray(edge_index[0], np.int64)
    dst = np.asarray(edge_index[1], np.int64)
    ea = np.asarray(edge_attr, np.float32).reshape(-1)

    cores = []
    for k in range(NCORES):
        sel = np.nonzero((dst >= k * NLOC) & (dst < (k + 1) * NLOC))[0]
        dl = dst[sel] - k * NLOC
        order = np.argsort(dl, kind="stable")
        cores.append((src[sel][order], dl[order], ea[sel][order]))

    nch = []
    for b in range(NBLK):
        lo, hi = b * P, min((b + 1) * P, NLOC)
        mx = max(int(np.count_nonzero((dl >= lo) & (dl < hi)))
                 for _, dl, _ in cores)
        nch.append(max(1, -(-mx // P)))
    totch = sum(nch)

    per_core = []
    for k in range(NCORES):
        s_k, dl_k, ea_k = cores[k]
        src_pad = np.zeros(totch * P, np.int64)
        dst_loc = np.zeros(totch * P, np.int64)
        dst_rel = np.full(totch * P, -1.0, np.float32)
        ea_pad = np.zeros(totch * P, np.float32)
        base = 0
        for b in range(NBLK):
            lo, hi = b * P, min((b + 1) * P, NLOC)
            m = (dl_k >= lo) & (dl_k < hi)
            cnt = int(np.count_nonzero(m))
            sl = slice(base * P, base * P + cnt)
            sp = s_k[m]
            src_pad[sl] = (sp // NLOC) * PPC + sp % NLOC
            dst_loc[sl] = dl_k[m]
            dst_rel[sl] = (dl_k[m] - lo).astype(np.float32)
            ea_pad[sl] = ea_k[m]
            base += nch[b]
        per_core.append({
            "src_i": _pack_idxs(src_pad),
            "dstl_i": _pack_idxs(dst_loc),
            "dst_rel": _col_layout(dst_rel).astype(np.float32),
            "dst_rel_row": dst_rel.astype(np.float32)[None, :],
            "ea_row": ea_pad.astype(ml_dtypes.bfloat16)[None, :],
            "ea_col": _col_layout(ea_pad).astype(ml_dtypes.bfloat16),
        })
    return nch, totch, per_core


# ------------------------------------------------------------ program build
def build_program(nch, totch, nz, sim_safe=False, nlayers=L, edge_phase=True, do_coll=True):
    """nz: dict of bools enabling the optional bias/gain paths.
    sim_safe: express leaky_relu via Abs (CoreSim lacks Lrelu)."""
    nchmax = max(nch)
    ncols = totch * 8
    nc = bacc.Bacc()

    x1_full = nc.declare_dram_parameter("x1_full", [NPAD, D], BF16, isOutput=False)
    x1_b16 = nc.declare_dram_parameter("x1_b16", [PPC, D], BF16, isOutput=False)
    x_loc = nc.declare_dram_parameter("x_loc", [NLOC, D], F32, isOutput=False)
    w_l = nc.declare_dram_parameter("w_l", [L, D, D], BF16, isOutput=False)
    w_r = nc.declare_dram_parameter("w_r", [L, D, D], BF16, isOutput=False)
    src_i = nc.declare_dram_parameter("src_i", [P, ncols], I16, isOutput=False)
    dstl_i = nc.declare_dram_parameter("dstl_i", [P, ncols], I16, isOutput=False)
    dst_rel = nc.declare_dram_parameter("dst_rel", [P, totch], F32, isOutput=False)
    dst_rel_row = nc.declare_dram_parameter("dst_rel_row", [1, totch * P], F32, isOutput=False)
    ea_row = nc.declare_dram_parameter("ea_row", [1, totch * P], BF16, isOutput=False)
    ea_col = nc.declare_dram_parameter("ea_col", [P, totch], BF16, isOutput=False)
    we_rep = nc.declare_dram_parameter("we_rep", [L, P, nchmax * D], BF16, isOutput=False)
    att_rep = nc.declare_dram_parameter("att_rep", [L, P, nchmax * D], BF16, isOutput=False)
    iota_t = nc.declare_dram_parameter("iota_t", [P, P], F32, isOutput=False)
    iota_c = nc.declare_dram_parameter("iota_c", [P, 1], F32, isOutput=False)
    b_lr = nc.declare_dram_parameter("b_lr", [L, 2, D], BF16, isOutput=False)
    b_out = nc.declare_dram_parameter("b_out", [L, P, D], F32, isOutput=False)
    ln_gb = nc.declare_dram_parameter("ln_gb", [L, 2, P, D], F32, isOutput=False)
    out_x = nc.declare_dram_parameter("out_x", [NLOC, D], F32, isOutput=True)

    xl_dram = nc.dram_tensor("xl_dram", [NPAD, D], BF16)
    xr_dram = nc.dram_tensor("xr_dram", [PPC, D], BF16)
    x2_loc = nc.dram_tensor("x2_loc", [NLOC, D], F32)
    x2_b16 = nc.dram_tensor("x2_b16", [PPC, D], BF16)
    x2_full = nc.dram_tensor("x2_full", [NPAD, D], BF16, addr_space="Shared")

    NT = NPAD // P    # 160 xl row tiles
    NTR = PPC // P    # 20 xr row tiles

    with tile.TileContext(nc) as tc:
      with tc.tile_pool(name="consts", bufs=1) as cp:
        iota_sb = cp.tile([P, P], F32)
        nc.sync.dma_start(iota_sb[:], iota_t[:, :])
        dst_rel_sb = cp.tile([P, totch], F32)
        nc.sync.dma_start(dst_rel_sb[:], dst_rel[:, :])
        ea_sb = cp.tile([P, totch], BF16)
        nc.sync.dma_start(ea_sb[:], ea_col[:, :])
        srci_sb = cp.tile([P, ncols], I16)
        nc.gpsimd.dma_start(srci_sb[:], src_i[:, :])
        ones_row = cp.tile([1, P], F32)
        nc.gpsimd.memset(ones_row[:], 1.0)
        iota_col = cp.tile([P, 1], F32)
        nc.sync.dma_start(iota_col[:], iota_c[:, :])
        ident_sb = cp.tile([P, P], BF16)
        nc.vector.tensor_scalar(
            out=ident_sb[:], in0=iota_sb[:], scalar1=iota_col[:, 0:1],
            scalar2=None, op0=ALU.is_equal)

        for l in range(nlayers):
            xfull = x1_full if l == 0 else x2_full
            xloc16 = x1_b16 if l == 0 else x2_b16
            # ---------------- GEMM phase ----------------
            with tc.tile_pool(name=f"gemm_x{l}", bufs=1) as gx, \
                 tc.tile_pool(name=f"gemm_w{l}", bufs=1) as gw, \
                 tc.tile_pool(name=f"gemm_ps{l}", bufs=4, space="PSUM") as gps, \
                 tc.tile_pool(name=f"gemm_o{l}", bufs=4) as go:
                xT0 = gx.tile([P, NPAD], BF16, tag="xT0")
                xT1 = gx.tile([P, NPAD], BF16, tag="xT1")
                nc.sync.dma_start(xT0[:], xfull[:, 0:P], transpose=True)
                nc.sync.dma_start(xT1[:], xfull[:, P:D], transpose=True)
                xl0 = gx.tile([P, PPC], BF16, tag="xl0")
                xl1 = gx.tile([P, PPC], BF16, tag="xl1")
                nc.sync.dma_start(xl0[:], xloc16[:, 0:P], transpose=True)
                nc.sync.dma_start(xl1[:], xloc16[:, P:D], transpose=True)
                wl0 = gw.tile([P, D], BF16, tag="wl0")
                wl1 = gw.tile([P, D], BF16, tag="wl1")
                wr0 = gw.tile([P, D], BF16, tag="wr0")
                wr1 = gw.tile([P, D], BF16, tag="wr1")
                nc.sync.dma_start(wl0[:], w_l[l, 0:P, :])
                nc.sync.dma_start(wl1[:], w_l[l, P:D, :])
                nc.sync.dma_start(wr0[:], w_r[l, 0:P, :])
                nc.sync.dma_start(wr1[:], w_r[l, P:D, :])
                if nz["b_lr"]:
                    ones_c = gw.tile([1, D], BF16, tag="ones_c")
                    nc.gpsimd.memset(ones_c[:], 1.0)
                    blr_sb = gw.tile([2, D], BF16, tag="blr_sb")
                    nc.sync.dma_start(blr_sb[:], b_lr[l, :, :])

                def gemm_quad(dst_dram, t4, ntile, a0, a1, w0, w1, bias_row):
                    # 4 row-tiles -> one SBUF tile -> one DMA
                    gq = min(4, ntile - t4 * 4)
                    ot = go.tile([P, 4, D], BF16, tag="g_o")
                    for j in range(gq):
                        t = t4 * 4 + j
                        ps = gps.tile([P, D], F32, space="PSUM", tag="g_ps")
                        nc.tensor.matmul(out=ps[:],
                                         lhsT=a0[:, t * P:(t + 1) * P],
                                         rhs=w0[:], start=True, stop=False)
                        nc.tensor.matmul(out=ps[:],
                                         lhsT=a1[:, t * P:(t + 1) * P],
                                         rhs=w1[:], start=False,
                                         stop=bias_row is None)
                        if bias_row is not None:
                            nc.tensor.matmul(out=ps[:], lhsT=ones_c[:, 0:1],
                                             rhs=bias_row, start=False,
                                             stop=True)
                        nc.any.tensor_copy(ot[:, j, :], ps[:])
                    nc.sync.dma_start(
                        dst_dram[t4 * 4 * P:t4 * 4 * P + gq * P, :]
                        .rearrange("(t p) d -> p t d", p=P), ot[:, 0:gq, :])

                for t4 in range((NT + 3) // 4):
                    gemm_quad(xl_dram, t4, NT, xT0, xT1, wl0, wl1,
                              blr_sb[0:1, :] if nz["b_lr"] else None)
                for t4 in range((NTR + 3) // 4):
                    gemm_quad(xr_dram, t4, NTR, xl0, xl1, wr0, wr1,
                              blr_sb[1:2, :] if nz["b_lr"] else None)

            tc.strict_bb_all_engine_barrier()
            if not edge_phase:
                continue
            # ---------------- edge phase ----------------
            with tc.tile_pool(name=f"edg{l}", bufs=2) as ep, \
                 tc.tile_pool(name=f"edg_s{l}", bufs=2) as es, \
                 tc.tile_pool(name=f"edg_ps{l}", bufs=2, space="PSUM") as eps, \
                 tc.tile_pool(name=f"blk_ps{l}", bufs=1, space="PSUM") as bps, \
                 tc.tile_pool(name=f"epi{l}", bufs=2) as epi, \
                 tc.tile_pool(name=f"lcon{l}", bufs=1) as lc:
                we_sb = lc.tile([1, D], BF16)
                nc.sync.dma_start(we_sb[:], we_rep[l, 0:1, 0:D])
                att_sb = lc.tile([P, nchmax * D], BF16)
                nc.sync.dma_start(att_sb[:], att_rep[l, :, :])
                if nz["b_out"]:
                    bout_sb = lc.tile([P, D], F32)
                    nc.sync.dma_start(bout_sb[:], b_out[l, :, :])
                if nz["ln_gb"]:
                    lng_sb = lc.tile([P, D], F32)
                    nc.sync.dma_start(lng_sb[:], ln_gb[l, 0, :, :])
                    lnb_sb = lc.tile([P, D], F32)
                    nc.sync.dma_start(lnb_sb[:], ln_gb[l, 1, :, :])

                cbase = 0
                for b in range(NBLK):
                    nchb = nch[b]
                    nn = min(P, NLOC - b * P)      # valid rows this block
                    fd = nchb * D                  # batched free size
                    nidx = nchb * P
                    icol = slice(cbase * 8, (cbase + nchb) * 8)

                    xl_g = ep.tile([P, nchmax, D], BF16, tag="xl_g")
                    nc.gpsimd.dma_gather(
                        xl_g[:, :nchb, :], xl_dram[:, :],
                        srci_sb[:, icol], nidx, nidx, D,
                        single_packet=False)
                    xr_blk = ep.tile([P, D], BF16, tag="xr_blk")
                    nc.sync.dma_start(xr_blk[:], xr_dram[b * P:(b + 1) * P, :])
                    drow = ep.tile([1, nchmax * P], F32, tag="drow")
                    nc.sync.dma_start(
                        drow[0:1, 0:nidx],
                        dst_rel_row[0:1, cbase * P:cbase * P + nidx])
                    earow = ep.tile([1, nchmax * P], BF16, tag="earow")
                    nc.sync.dma_start(
                        earow[0:1, 0:nidx],
                        ea_row[0:1, cbase * P:cbase * P + nidx])

                    # v[e,hc] = xr[dst_e,hc] + ea_e*We[hc] + xl_g[e,hc], on PE
                    # via Mt (one-hot dst, nodes-part) + rank-1 + identity.
                    m_t = es.tile([P, nchmax, D], BF16, tag="m_t")
                    ngrp = (nchb + 3) // 4
                    for g in range(ngrp):
                        gsz = min(4, nchb - g * 4)
                        gw = gsz * P
                        bc_ps = eps.tile([P, 4 * P], F32, space="PSUM",
                                         tag="bc_ps")
                        nc.tensor.matmul(
                            out=bc_ps[:, 0:gw], lhsT=ones_row[0:1, :],
                            rhs=drow[0:1, g * 4 * P:g * 4 * P + gw],
                            start=True, stop=True)
                        mt4 = es.tile([P, 4 * P], BF16, tag="mt4")
                        nc.vector.tensor_scalar(
                            out=mt4[:, 0:gw], in0=bc_ps[:, 0:gw],
                            scalar1=iota_col[:, 0:1], scalar2=None,
                            op0=ALU.is_equal)
                        v_ps = eps.tile([P, 4, D], F32, space="PSUM",
                                        tag="v_ps")
                        for cc in range(gsz):
                            c = g * 4 + cc
                            nc.tensor.matmul(
                                out=v_ps[:, cc, :],
                                lhsT=mt4[:, cc * P:(cc + 1) * P],
                                rhs=xr_blk[:], start=True, stop=False)
                            nc.tensor.matmul(
                                out=v_ps[:, cc, :],
                                lhsT=earow[0:1, c * P:(c + 1) * P],
                                rhs=we_sb[0:1, 0:D], start=False, stop=False)
                            nc.tensor.matmul(
                                out=v_ps[:, cc, :], lhsT=ident_sb[:],
                                rhs=xl_g[:, c, :], start=False, stop=True)
                        if sim_safe:
                            ab = es.tile([P, 4, D], BF16, tag="ab")
                            nc.scalar.activation(
                                ab[:, 0:gsz, :], v_ps[:, 0:gsz, :], AF.Abs,
                                scale=(1.0 - NEG_SLOPE) / 2.0)
                            sv = es.tile([P, 4, D], BF16, tag="sv")
                            nc.vector.tensor_scalar(
                                out=sv[:, 0:gsz, :], in0=v_ps[:, 0:gsz, :],
                                scalar1=(1.0 + NEG_SLOPE) / 2.0, scalar2=None,
                                op0=ALU.mult)
                            nc.vector.tensor_tensor(
                                out=m_t[:, g * 4:g * 4 + gsz, :],
                                in0=sv[:, 0:gsz, :], in1=ab[:, 0:gsz, :],
                                op=ALU.add)
                        else:
                            nc.scalar.activation(
                                m_t[:, g * 4:g * 4 + gsz, :], v_ps[:, 0:gsz, :],
                                AF.Prelu, alpha=NEG_SLOPE)
                    s_t = es.tile([P, nchmax, D], BF16, tag="s_t")
                    nc.vector.tensor_tensor(
                        out=s_t[:, :nchb, :], in0=m_t[:, :nchb, :],
                        in1=att_sb[:, 0:fd].rearrange("p (c d) -> p c d", d=D),
                        op=ALU.mult)
                    # logits[e, c, h] = sum_c32 s  -> exp -> bf16
                    logit = es.tile([P, nchmax, H], F32, tag="logit")
                    nc.vector.tensor_reduce(
                        out=logit[:, :nchb, :],
                        in_=s_t[:, :nchb, :].rearrange("p c (h w) -> p c h w", w=C),
                        axis=mybir.AxisListType.X, op=ALU.add)
                    ex_f = es.tile([P, nchmax, H], F32, tag="ex_f")
                    nc.scalar.activation(
                        ex_f[:, :nchb, :], logit[:, :nchb, :], AF.Exp)
                    ex_b = es.tile([P, nchmax, H], BF16, tag="ex_b")
                    nc.vector.tensor_copy(ex_b[:, :nchb, :], ex_f[:, :nchb, :])
                    # Xw = ex * xl_g
                    xw = es.tile([P, nchmax, D], BF16, tag="xw")
                    nc.vector.tensor_tensor(
                        out=xw[:, :nchb, :].rearrange("p c (h w) -> p c h w", w=C),
                        in0=xl_g[:, :nchb, :].rearrange("p c (h w) -> p c h w", w=C),
                        in1=ex_b[:, :nchb, :].unsqueeze(3).to_broadcast(
                            [P, nchb, H, C]),
                        op=ALU.mult)

                    den_ps = bps.tile([P, H], F32, space="PSUM", tag="den_ps")
                    u_ps = bps.tile([P, D], F32, space="PSUM", tag="u_ps")
                    for c in range(nchb):
                        m_oh = es.tile([P, P], BF16, tag="m_oh")
                        nc.vector.tensor_scalar(
                            out=m_oh[:], in0=iota_sb[:],
                            scalar1=dst_rel_sb[:, cbase + c:cbase + c + 1],
                            scalar2=None, op0=ALU.is_equal)
                        nc.tensor.matmul(out=den_ps[:], lhsT=m_oh[:],
                                         rhs=ex_b[:, c, :], start=(c == 0),
                                         stop=(c == nchb - 1))
                        nc.tensor.matmul(out=u_ps[:], lhsT=m_oh[:],
                                         rhs=xw[:, c, :], start=(c == 0),
                                         stop=(c == nchb - 1))

                    # out = U / den  (per node), then bias/LN/ELU/residual
                    den2 = epi.tile([P, H], F32, tag="den2")
                    nc.vector.tensor_scalar(
                        out=den2[:nn], in0=den_ps[:nn], scalar1=1e-16,
                        scalar2=None, op0=ALU.add)
                    drec = epi.tile([P, H], F32, tag="drec")
                    nc.vector.reciprocal(drec[:nn], den2[:nn])
                    outw = epi.tile([P, D], F32, tag="outw")
                    nc.vector.tensor_tensor(
                        out=outw[:nn].rearrange("p (h w) -> p h w", w=C),
                        in0=u_ps[:nn].rearrange("p (h w) -> p h w", w=C),
                        in1=drec[:nn].unsqueeze(2).to_broadcast([nn, H, C]),
                        op=ALU.mult)
                    if nz["b_out"]:
                        nc.vector.tensor_tensor(
                            out=outw[:nn], in0=outw[:nn], in1=bout_sb[:nn],
                            op=ALU.add)
                    # layernorm stats
                    ssum = epi.tile([P, 1], F32, tag="ssum")
                    nc.vector.tensor_reduce(
                        out=ssum[:nn], in_=outw[:nn],
                        axis=mybir.AxisListType.X, op=ALU.add)
                    nmu = epi.tile([P, 1], F32, tag="nmu")
                    nc.vector.tensor_scalar(
                        out=nmu[:nn], in0=ssum[:nn], scalar1=-1.0 / D,
                        scalar2=None, op0=ALU.mult)
                    sqj = epi.tile([P, D], F32, tag="sqj")
                    vsum = epi.tile([P, 1], F32, tag="vsum")
                    nc.scalar.activation(
                        sqj[:nn], outw[:nn], AF.Square, bias=nmu[:nn],
                        accum_out=vsum[:nn])
                    varr = epi.tile([P, 1], F32, tag="varr")
                    nc.vector.tensor_scalar(
                        out=varr[:nn], in0=vsum[:nn], scalar1=1.0 / D,
                        scalar2=LN_EPS, op0=ALU.mult, op1=ALU.add)
                    lnv = epi.tile([P, 1], F32, tag="lnv")
                    nc.scalar.activation(lnv[:nn], varr[:nn], AF.Ln)
                    isig = epi.tile([P, 1], F32, tag="isig")
                    nc.scalar.activation(isig[:nn], lnv[:nn], AF.Exp, scale=-0.5)
                    y_t = epi.tile([P, D], F32, tag="y_t")
                    nc.vector.tensor_scalar(
                        out=y_t[:nn], in0=outw[:nn], scalar1=nmu[:nn],
                        scalar2=isig[:nn], op0=ALU.add, op1=ALU.mult)
                    if nz["ln_gb"]:
                        nc.vector.tensor_tensor(
                            out=y_t[:nn], in0=y_t[:nn], in1=lng_sb[:nn], op=ALU.mult)
                        nc.vector.tensor_tensor(
                            out=y_t[:nn], in0=y_t[:nn], in1=lnb_sb[:nn], op=ALU.add)
                    # elu(y) = max(y,0) + min(exp(y),1) - 1
                    e_t = epi.tile([P, D], F32, tag="e_t")
                    nc.scalar.activation(e_t[:nn], y_t[:nn], AF.Exp)
                    a_t = epi.tile([P, D], F32, tag="a_t")
                    nc.vector.tensor_scalar(
                        out=a_t[:nn], in0=e_t[:nn], scalar1=1.0, scalar2=-1.0,
                        op0=ALU.min, op1=ALU.add)
                    r_t = epi.tile([P, D], F32, tag="r_t")
                    nc.vector.tensor_scalar(
                        out=r_t[:nn], in0=y_t[:nn], scalar1=0.0, scalar2=None,
                        op0=ALU.max)
                    xo_t = epi.tile([P, D], F32, tag="xo_t")
                    xres = x_loc if l == 0 else x2_loc
                    nc.sync.dma_start(xo_t[:nn, :],
                                      xres[b * P:b * P + nn, :])
                    nc.vector.tensor_tensor(
                        out=a_t[:nn], in0=a_t[:nn], in1=r_t[:nn], op=ALU.add)
                    xn_t = epi.tile([P, D], F32, tag="xn_t")
                    nc.vector.tensor_tensor(
                        out=xn_t[:nn], in0=a_t[:nn], in1=xo_t[:nn], op=ALU.add)
                    if l == 0:
                        xnb = epi.tile([P, D], BF16, tag="xnb")
                        nc.vector.tensor_copy(xnb[:nn], xn_t[:nn])
                        nc.sync.dma_start(x2_b16[b * P:b * P + nn, :],
                                          xnb[:nn, :])
                        nc.sync.dma_start(x2_loc[b * P:b * P + nn, :],
                                          xn_t[:nn, :])
                    else:
                        nc.sync.dma_start(out_x[b * P:b * P + nn, :],
                                          xn_t[:nn, :])
                    cbase += nchb

            if l == 0 and do_coll:
                tc.strict_bb_all_engine_barrier()
                # zero pad rows of x2_b16 beyond NLOC before the allgather
                with tc.tile_pool(name="padz", bufs=1) as pz:
                    zt = pz.tile([P, D], BF16)
                    nc.gpsimd.memset(zt[:], 0.0)
                    for r in range(NLOC, PPC, P):
                        rows = min(P, PPC - r)
                        nc.sync.dma_start(x2_b16[r:r + rows, :], zt[:rows, :])
                    nc.gpsimd.collective_compute(
                        "AllGather", ALU.bypass,
                        replica_groups=[list(range(NCORES))],
                        ins=[x2_b16[:, :]], outs=[x2_full[:, :]])
                tc.strict_bb_all_engine_barrier()

    nc.compile()
    return nc


# ---------------------------------------------------------------- interface
_BF = ml_dtypes.bfloat16


def _to_bf16(a):
    return np.asarray(a, np.float32).astype(_BF)


def kernel(x, edge_index, edge_attr, Wl, bl, Wr, br, We, att, bias_out,
           ln_g, ln_b, trace=False):
    x = np.asarray(x, np.float32)
    Wl = np.asarray(Wl, np.float32)
    Wr = np.asarray(Wr, np.float32)
    We = np.asarray(We, np.float32)
    att = np.asarray(att, np.float32)
    bl = np.asarray(bl, np.float32)
    br = np.asarray(br, np.float32)
    bias_out = np.asarray(bias_out, np.float32)
    ln_g = np.asarray(ln_g, np.float32)
    ln_b = np.asarray(ln_b, np.float32)

    nch, totch, per_core = _prep_edges(edge_index, edge_attr)
    nchmax = max(nch)

    nz = {
        "b_lr": bool(np.any(bl) or np.any(br)),
        "b_out": bool(np.any(bias_out)),
        "ln_gb": bool(np.any(ln_g != 1.0) or np.any(ln_b)),
    }
    nc = build_program(nch, totch, nz, sim_safe=(os.environ.get("GAT_SIMSAFE","0")=="1"))

    # replicated inputs
    x_pad = np.zeros((NPAD, D), _BF)
    xv = x.reshape(NCORES, NLOC, D)
    for k in range(NCORES):
        x_pad[k * PPC:k * PPC + NLOC] = _to_bf16(xv[k])
    we_rep = np.zeros((L, P, nchmax * D), _BF)
    att_rep = np.zeros((L, P, nchmax * D), _BF)
    for l in range(L):
        we_rep[l] = np.tile(_to_bf16(We[l, 0]), (P, nchmax))
        att_rep[l] = np.tile(_to_bf16(att[l].reshape(D)), (P, nchmax))
    iota_np = np.tile(np.arange(P, dtype=np.float32), (P, 1))
    b_lr_np = np.stack([_to_bf16(bl), _to_bf16(br)], axis=1)  # [L, 2, D]
    b_out_np = np.tile(bias_out[:, None, :], (1, P, 1)).astype(np.float32)
    ln_gb_np = np.stack(
        [np.tile(ln_g[:, None, :], (1, P, 1)),
         np.tile(ln_b[:, None, :], (1, P, 1))], axis=1).astype(np.float32)

    shared = {
        "x1_full": x_pad, "w_l": _to_bf16(Wl), "w_r": _to_bf16(Wr),
        "we_rep": we_rep, "att_rep": att_rep, "iota_t": iota_np,
        "b_lr": b_lr_np, "b_out": b_out_np, "ln_gb": ln_gb_np,
        "iota_c": np.arange(P, dtype=np.float32)[:, None],
    }
    in_maps = []
    for k in range(NCORES):
        m = dict(shared)
        m.update(per_core[k])
        m["x_loc"] = np.ascontiguousarray(xv[k])
        x1b = np.zeros((PPC, D), _BF)
        x1b[:NLOC] = _to_bf16(xv[k])
        m["x1_b16"] = x1b
        in_maps.append(m)

    res = run_bass_kernel_spmd(nc, in_maps, list(range(NCORES)), trace=trace)
    out = np.concatenate([res.results[k]["out_x"] for k in range(NCORES)], 0)
    if trace:
        kernel.last_exec_time_ns = res.exec_time_ns
    return out



# revision 13
# speedup vs baseline: 1.4954x; 1.4954x over previous
"""DomainAwareGAT (2-layer GATv2 + LN + ELU + residual) on 8 Trainium2 cores.

v2 strategy (edge-sharded by destination, one core owns dst rows
[k*2500, (k+1)*2500)):
 - Node features are kept TRANSPOSED (x^T) end to end: layer-0 x^T comes
   from the host, the epilogue produces x2^T via PE transposes, and the
   inter-layer AllGather ships x2^T — no DMA transposes anywhere.
 - xl = x@Wl is computed replicated per core from streamed x^T tiles and
   written to a tile-major DRAM table (4KB-per-partition descriptors);
   per-edge rows are fetched with dma_gather using tile-major row ids.
 - xr is computed only for local nodes in 125-row tiles and kept
   SBUF-resident, with We pre-planted in partition 126 and zeros in
   125/127 so the per-chunk one-hot matmul adds xr[dst] AND ea*We in a
   single PE op (lhsT = [one-hot dst | ea | 0] built by PE-transposing
   the edge-side one-hot).
 - The edge-side one-hot (m_oh) for a whole block is built by ONE
   tensor_tensor is_equal against a constant iota tile (bf16), avoiding
   the slow per-chunk vector-scalar ops.
 - den and u are accumulated by a single matmul per chunk over an
   [ex | ex*xl] rhs (N=264).
 - LayerNorm/ELU epilogue uses activation(scale=isig, bias=-mu*isig)
   and scalar_tensor_tensor to avoid slow two-vector-scalar DVE ops;
   the residual is kept as (x - shift) so ELU's -1 folds away.
 - The AllGather overlaps the layer-1 xr GEMM (which reads local x2^T
   straight from SBUF).
"""
import os
import sys

sys.path.insert(0, "/opt/trn_rl_repo")

import numpy as np
import ml_dtypes

import concourse.bass as bass
import concourse.tile as tile
from concourse import bacc, mybir
from concourse.bass_utils import run_bass_kernel_spmd

F32 = mybir.dt.float32
BF16 = mybir.dt.bfloat16
I16 = mybir.dt.int16
AF = mybir.ActivationFunctionType
ALU = mybir.AluOpType

N, E, D, H, C, L = 20000, 320000, 256, 8, 32, 2
NEG_SLOPE = 0.2
LN_EPS = 1e-5
NCORES = 8
NLOC = N // NCORES            # 2500 nodes per core
BLK = 125                     # dst nodes per block (125*20 = 2500)
NBLK = NLOC // BLK            # 20 blocks per core
P = 128
PPC = 2560                    # padded nodes per core (20 x 128)
NT = NCORES * PPC // P        # 160 tiles in the global gather table
NPAD = NCORES * PPC


# ---------------------------------------------------------------- host prep
def _pack_idxs(e_list):
    """Pack a flat gather-index list into dma_gather's [128, n/16] layout:
    out[p, c, :] = table[e_list[c*128 + p]]."""
    nch = len(e_list) // P
    e3 = np.asarray(e_list, np.int16).reshape(nch, 8, 16)  # [c, g, a]
    return np.tile(e3.transpose(2, 0, 1).reshape(16, nch * 8), (8, 1))


def _prep_edges(edge_index, edge_attr):
    """Bucket edges by dst core, sort by dst, pad to per-block chunk counts
    shared by all cores (SPMD). Gather rows are tile-major remapped."""
    src = np.asarray(edge_index[0], np.int64)
    dst = np.asarray(edge_index[1], np.int64)
    ea = np.asarray(edge_attr, np.float32).reshape(-1)

    cores = []
    for k in range(NCORES):
        sel = np.nonzero((dst >= k * NLOC) & (dst < (k + 1) * NLOC))[0]
        dl = dst[sel] - k * NLOC
        order = np.argsort(dl, kind="stable")
        cores.append((src[sel][order], dl[order], ea[sel][order]))

    nch = []
    for b in range(NBLK):
        lo, hi = b * BLK, (b + 1) * BLK
        mx = max(int(np.count_nonzero((dl >= lo) & (dl < hi)))
                 for _, dl, _ in cores)
        nch.append(max(1, -(-mx // P)))
    totch = sum(nch)

    per_core = []
    for k in range(NCORES):
        s_k, dl_k, ea_k = cores[k]
        src_pad = np.zeros(totch * P, np.int64)
        dst_rel = np.full(totch * P, -1.0, np.float32)
        ea_pad = np.zeros(totch * P, np.float32)
        base = 0
        for b in range(NBLK):
            lo, hi = b * BLK, (b + 1) * BLK
            m = (dl_k >= lo) & (dl_k < hi)
            cnt = int(np.count_nonzero(m))
            sl = slice(base * P, base * P + cnt)
            sp = s_k[m]                        # global node ids
            ck = sp // NLOC                    # owner core of source
            ii = sp % NLOC                     # local id on owner
            # tile-major row id in the [NT*128, D] gather table
            src_pad[sl] = (ii % P) * NT + ck * (PPC // P) + ii // P
            dst_rel[sl] = (dl_k[m] - lo).astype(np.float32)
            ea_pad[sl] = ea_k[m]
            base += nch[b]
        per_core.append({
            "src_i": _pack_idxs(src_pad),
            "dst_rel": np.ascontiguousarray(
                dst_rel.reshape(-1, P).T).astype(ml_dtypes.bfloat16),
            "ea_row": ea_pad.astype(ml_dtypes.bfloat16)[None, :],
        })
    return nch, totch, per_core


# ------------------------------------------------------------ program build
def build_program(nch, totch, nz, sim_safe=False):
    nchmax = max(nch)
    nc = bacc.Bacc()

    # --- inputs
    x1t = nc.declare_dram_parameter("x1t", [P, 2, NCORES, PPC], BF16, isOutput=False)
    x1t_loc = nc.declare_dram_parameter("x1t_loc", [P, 2, NLOC], BF16, isOutput=False)
    xloc_m2 = nc.declare_dram_parameter("xloc_m2", [P, NBLK, D], F32, isOutput=False)
    w_l = nc.declare_dram_parameter("w_l", [L, D, D], BF16, isOutput=False)
    w_r = nc.declare_dram_parameter("w_r", [L, D, D], BF16, isOutput=False)
    src_i = nc.declare_dram_parameter("src_i", [P, totch * 8], I16, isOutput=False)
    dst_rel = nc.declare_dram_parameter("dst_rel", [P, totch], BF16, isOutput=False)
    ea_row = nc.declare_dram_parameter("ea_row", [1, totch * P], BF16, isOutput=False)
    we_rep = nc.declare_dram_parameter("we_rep", [L, 1, NBLK * D], BF16, isOutput=False)
    att_rep = nc.declare_dram_parameter("att_rep", [L, P, nchmax * D], BF16, isOutput=False)
    iota_big = nc.declare_dram_parameter("iota_big", [P, nchmax * P], BF16, isOutput=False)
    ident_b = nc.declare_dram_parameter("ident_b", [P, P], BF16, isOutput=False)
    ident_f = nc.declare_dram_parameter("ident_f", [P, P], F32, isOutput=False)
    b_lr = nc.declare_dram_parameter("b_lr", [L, 2, D], F32, isOutput=False)
    b_out = nc.declare_dram_parameter("b_out", [L, 1, D], F32, isOutput=False)
    ln_gb = nc.declare_dram_parameter("ln_gb", [L, 2, D], F32, isOutput=False)
    out_x = nc.declare_dram_parameter("out_x", [NLOC, D], F32, isOutput=True)

    # --- internal dram
    xl_dram = nc.dram_tensor("xl_dram", [P, NT, D], BF16)
    x2t_loc = nc.dram_tensor("x2t_loc", [2, P, PPC], BF16)
    x2t_full = nc.dram_tensor("x2t_full", [NCORES, 2, P, PPC], BF16,
                              addr_space="Shared")

    xl_view = xl_dram[:, :, :].rearrange("p t d -> (p t) d")

    def x1t_chunk(k, off):
        return x1t[:, :, k, off:off + 512]

    def x2t_chunk(k, off):
        return x2t_full[k, :, :, off:off + 512].rearrange("d p c -> p d c")

    NCHUNK = NPAD // 512      # 40 GEMM column chunks of 512 nodes

    with tile.TileContext(nc) as tc:
      with tc.tile_pool(name="consts", bufs=1) as cp:
        ident_sb = cp.tile([P, P], BF16)
        nc.sync.dma_start(ident_sb[:], ident_b[:, :])
        identf_sb = cp.tile([P, P], F32)
        nc.sync.dma_start(identf_sb[:], ident_f[:, :])
        iota_sb = cp.tile([P, nchmax, P], BF16)
        nc.sync.dma_start(
            iota_sb[:].rearrange("p c j -> p (c j)"), iota_big[:, :])
        dstrel_sb = cp.tile([P, totch], BF16)
        nc.sync.dma_start(dstrel_sb[:], dst_rel[:, :])
        srci_sb = cp.tile([P, totch * 8], I16)
        nc.gpsimd.dma_start(srci_sb[:], src_i[:, :])
        xres_sb = cp.tile([P, NBLK, D], F32, tag="xres0")
        nc.sync.dma_start(xres_sb[:], xloc_m2[:, :, :])
        x2res_sb = cp.tile([P, NBLK, D], F32, tag="xres1")
        x2t_sb = cp.tile([P, 2, PPC], BF16, tag="x2t_sb")
        if nz["b_out"]:
            bout_r = cp.tile([1, D], F32)
            nc.sync.dma_start(bout_r[:], b_out[0, :, :])  # per-layer reload below
        if nz["ln_gb"]:
            lng_bc = cp.tile([P, D], F32, tag="lng")
            lnb_bc = cp.tile([P, D], F32, tag="lnb")

        # zero the pad columns of x2t_sb once (cols 2500..2559)
        nc.vector.memset(x2t_sb[:, :, NLOC:PPC], 0.0)

        def xr_gemm(l, lhsT_of):
            """xr for local nodes into a fresh resident tile; returns it.
            lhsT_of(dt, t) -> [128, 125] SBUF AP of local x^T."""
            with tc.tile_pool(name=f"xrw{l}", bufs=1) as gw, \
                 tc.tile_pool(name=f"xrp{l}", bufs=4, space="PSUM") as gp:
                wr0 = gw.tile([P, D], BF16, tag="wr0")
                wr1 = gw.tile([P, D], BF16, tag="wr1")
                nc.sync.dma_start(wr0[:], w_r[l, 0:P, :])
                nc.sync.dma_start(wr1[:], w_r[l, P:D, :])
                xr_res = cp.tile([P, NBLK, D], BF16, tag=f"xr{l}")
                nc.vector.memset(xr_res[:], 0.0)
                nc.sync.dma_start(
                    xr_res[127:128, :, :].rearrange("p t d -> p (t d)"),
                    we_rep[l, :, :])
                for t in range(NBLK):
                    ps = gp.tile([P, D], F32, space="PSUM", tag="xr_ps")
                    nc.tensor.matmul(out=ps[0:BLK, :], lhsT=lhsT_of(0, t),
                                     rhs=wr0[:], start=True, stop=False)
                    nc.tensor.matmul(out=ps[0:BLK, :], lhsT=lhsT_of(1, t),
                                     rhs=wr1[:], start=False, stop=True)
                    nc.any.tensor_copy(xr_res[0:BLK, t, :], ps[0:BLK, :])
                if nz["b_lr"]:
                    brb = gw.tile([P, D], F32, tag="brb")
                    nc.gpsimd.partition_broadcast(brb[:], b_lr[l, 1:2, :])
                    brb16 = gw.tile([P, D], BF16, tag="brb16")
                    nc.vector.tensor_copy(brb16[:], brb[:])
                    for t in range(NBLK):
                        nc.vector.tensor_tensor(
                            out=xr_res[0:BLK, t, :], in0=xr_res[0:BLK, t, :],
                            in1=brb16[0:BLK, :], op=ALU.add)
                return xr_res

        def xl_gemm(l, xt_chunk):
            """xl for all nodes -> xl_dram (tile-major)."""
            with tc.tile_pool(name=f"xlw{l}", bufs=1) as gw, \
                 tc.tile_pool(name=f"xlx{l}", bufs=3) as gx, \
                 tc.tile_pool(name=f"xlp{l}", bufs=4, space="PSUM") as gp, \
                 tc.tile_pool(name=f"xlo{l}", bufs=3) as go:
                wl0 = gw.tile([P, D], BF16, tag="wl0")
                wl1 = gw.tile([P, D], BF16, tag="wl1")
                nc.sync.dma_start(wl0[:], w_l[l, 0:P, :])
                nc.sync.dma_start(wl1[:], w_l[l, P:D, :])
                if nz["b_lr"]:
                    blb = gw.tile([P, D], F32, tag="blb")
                    nc.gpsimd.partition_broadcast(blb[:], b_lr[l, 0:1, :])
                for j in range(NCHUNK):
                    k, off = j // 5, (j % 5) * 512
                    xt = gx.tile([P, 2, 512], BF16, tag="xt")
                    nc.sync.dma_start(xt[:], xt_chunk(k, off))
                    ot = go.tile([P, 4, D], BF16, tag="ot")
                    for tt in range(4):
                        ps = gp.tile([P, D], F32, space="PSUM", tag="xl_ps")
                        nc.tensor.matmul(
                            out=ps[:], lhsT=xt[:, 0, tt * P:(tt + 1) * P],
                            rhs=wl0[:], start=True, stop=False)
                        nc.tensor.matmul(
                            out=ps[:], lhsT=xt[:, 1, tt * P:(tt + 1) * P],
                            rhs=wl1[:], start=False, stop=True)
                        if nz["b_lr"]:
                            nc.vector.scalar_tensor_tensor(
                                out=ot[:, tt, :], in0=ps[:], scalar=1.0,
                                in1=blb[:], op0=ALU.mult, op1=ALU.add)
                        else:
                            nc.any.tensor_copy(ot[:, tt, :], ps[:])
                    nc.sync.dma_start(xl_dram[:, j * 4:(j + 1) * 4, :], ot[:])

        def edge_phase(l, xr_res, att_l):
            cbase = 0
            with tc.tile_pool(name=f"eg{l}", bufs=2) as ep, \
                 tc.tile_pool(name=f"es{l}", bufs=2) as es, \
                 tc.tile_pool(name=f"ev{l}", bufs=2, space="PSUM") as evp, \
                 tc.tile_pool(name=f"em{l}", bufs=1, space="PSUM") as emp, \
                 tc.tile_pool(name=f"eu{l}", bufs=2, space="PSUM") as eup, \
                 tc.tile_pool(name=f"et{l}", bufs=1, space="PSUM") as etp, \
                 tc.tile_pool(name=f"epi{l}", bufs=2) as epi:
                for b in range(NBLK):
                    nchb = nch[b]
                    nidx = nchb * P
                    icol = slice(cbase * 8, (cbase + nchb) * 8)

                    xl_g = ep.tile([P, nchmax, D], BF16, tag="xl_g")
                    nc.gpsimd.dma_gather(
                        xl_g[:, :nchb, :], xl_view,
                        srci_sb[:, icol], nidx, nidx, D,
                        single_packet=False)

                    # edge-side one-hot for the whole block: one IS_EQ
                    m_oh = es.tile([P, nchmax, P], BF16, tag="m_oh")
                    nc.vector.tensor_tensor(
                        out=m_oh[:, :nchb, :], in0=iota_sb[:, :nchb, :],
                        in1=dstrel_sb[:, cbase:cbase + nchb].unsqueeze(2)
                            .to_broadcast([P, nchb, P]),
                        op=ALU.is_equal)
                    # node-side one-hot (+ea row at partition 127; xr_res
                    # carries We at 127 and zeros at 125/126)
                    mt_all = es.tile([P, nchmax, P], BF16, tag="mt_all")
                    nc.sync.dma_start(
                        mt_all[127:128, 0:nchb, :].rearrange("p c j -> p (c j)"),
                        ea_row[0:1, cbase * P:cbase * P + nidx])
                    for c in range(nchb):
                        mt_ps = emp.tile([P, P], BF16, space="PSUM", tag="mt_ps")
                        nc.tensor.transpose(mt_ps[:], m_oh[:, c, :], ident_sb[:])
                        nc.scalar.copy(mt_all[0:127, c, :], mt_ps[0:127, :])

                    xw = es.tile([P, nchmax, 264], BF16, tag="xw")
                    ngrp = (nchb + 3) // 4
                    for g in range(ngrp):
                        gsz = min(4, nchb - g * 4)
                        c0 = g * 4
                        v_ps = evp.tile([P, 4, D], F32, space="PSUM", tag="v_ps")
                        for cc in range(gsz):
                            c = c0 + cc
                            nc.tensor.matmul(
                                out=v_ps[:, cc, :], lhsT=mt_all[:, c, :],
                                rhs=xr_res[:, b, :], start=True, stop=False)
                            nc.tensor.matmul(
                                out=v_ps[:, cc, :], lhsT=ident_sb[:],
                                rhs=xl_g[:, c, :], start=False, stop=True)
                        m_t = es.tile([P, 4, D], BF16, tag="m_t")
                        if sim_safe:
                            ab = es.tile([P, 4, D], BF16, tag="ab")
                            nc.scalar.activation(
                                ab[:, 0:gsz, :], v_ps[:, 0:gsz, :], AF.Abs,
                                scale=(1.0 - NEG_SLOPE) / 2.0)
                            sv = es.tile([P, 4, D], BF16, tag="sv")
                            nc.vector.tensor_scalar(
                                out=sv[:, 0:gsz, :], in0=v_ps[:, 0:gsz, :],
                                scalar1=(1.0 + NEG_SLOPE) / 2.0, scalar2=None,
                                op0=ALU.mult)
                            nc.vector.tensor_tensor(
                                out=m_t[:, 0:gsz, :], in0=sv[:, 0:gsz, :],
                                in1=ab[:, 0:gsz, :], op=ALU.add)
                        else:
                            nc.scalar.activation(
                                m_t[:, 0:gsz, :], v_ps[:, 0:gsz, :],
                                AF.Prelu, alpha=NEG_SLOPE)
                        s_t = es.tile([P, 4, D], BF16, tag="s_t")
                        nc.vector.tensor_tensor(
                            out=s_t[:, 0:gsz, :], in0=m_t[:, 0:gsz, :],
                            in1=att_l[:, c0 * D:(c0 + gsz) * D]
                                .rearrange("p (c d) -> p c d", d=D),
                            op=ALU.mult)
                        logit = epi.tile([P, 4, H], F32, tag="logit")
                        nc.vector.tensor_reduce(
                            out=logit[:, 0:gsz, :],
                            in_=s_t[:, 0:gsz, :].rearrange(
                                "p c (h w) -> p c h w", w=C),
                            axis=mybir.AxisListType.X, op=ALU.add)
                        nc.scalar.activation(
                            xw[:, c0:c0 + gsz, 0:H], logit[:, 0:gsz, :], AF.Exp)
                        nc.vector.tensor_tensor(
                            out=xw[:, c0:c0 + gsz, 8:264].rearrange(
                                "p c (h w) -> p c h w", w=C),
                            in0=xl_g[:, c0:c0 + gsz, :].rearrange(
                                "p c (h w) -> p c h w", w=C),
                            in1=xw[:, c0:c0 + gsz, 0:H].unsqueeze(3)
                                .to_broadcast([P, gsz, H, C]),
                            op=ALU.mult)

                    u_ps = eup.tile([P, 264], F32, space="PSUM", tag="u_ps")
                    for c in range(nchb):
                        nc.tensor.matmul(
                            out=u_ps[:], lhsT=m_oh[:, c, :], rhs=xw[:, c, :],
                            start=(c == 0), stop=(c == nchb - 1))

                    # ---------------- per-node epilogue (125 rows) ----------
                    den2 = epi.tile([P, H], F32, tag="den2")
                    nc.vector.tensor_scalar(
                        out=den2[:BLK], in0=u_ps[:BLK, 0:H], scalar1=1e-16,
                        scalar2=None, op0=ALU.add)
                    drec = epi.tile([P, H], F32, tag="drec")
                    nc.vector.reciprocal(drec[:BLK], den2[:BLK])
                    outw = epi.tile([P, D], F32, tag="outw")
                    nc.vector.tensor_tensor(
                        out=outw[:BLK].rearrange("p (h w) -> p h w", w=C),
                        in0=u_ps[:BLK, 8:264].rearrange("p (h w) -> p h w", w=C),
                        in1=drec[:BLK].unsqueeze(2).to_broadcast([BLK, H, C]),
                        op=ALU.mult)
                    if nz["b_out"]:
                        nc.vector.tensor_tensor(
                            out=outw[:BLK], in0=outw[:BLK],
                            in1=bout_bc[:BLK], op=ALU.add)
                    ssum = epi.tile([P, 1], F32, tag="ssum")
                    nc.vector.tensor_reduce(
                        out=ssum[:BLK], in_=outw[:BLK],
                        axis=mybir.AxisListType.X, op=ALU.add)
                    nmu = epi.tile([P, 1], F32, tag="nmu")
                    nc.vector.tensor_scalar(
                        out=nmu[:BLK], in0=ssum[:BLK], scalar1=-1.0 / D,
                        scalar2=None, op0=ALU.mult)
                    sqj = epi.tile([P, D], F32, tag="sqj")
                    vsum = epi.tile([P, 1], F32, tag="vsum")
                    nc.scalar.activation(
                        sqj[:BLK], outw[:BLK], AF.Square, bias=nmu[:BLK],
                        accum_out=vsum[:BLK])
                    varr = epi.tile([P, 1], F32, tag="varr")
                    nc.vector.tensor_scalar(
                        out=varr[:BLK], in0=vsum[:BLK], scalar1=1.0 / D,
                        scalar2=LN_EPS, op0=ALU.mult, op1=ALU.add)
                    rvar = epi.tile([P, 1], F32, tag="rvar")
                    nc.vector.reciprocal(rvar[:BLK], varr[:BLK])
                    isig = epi.tile([P, 1], F32, tag="isig")
                    nc.scalar.sqrt(isig[:BLK], rvar[:BLK])
                    nmi = epi.tile([P, 1], F32, tag="nmi")
                    nc.vector.tensor_tensor(
                        out=nmi[:BLK], in0=nmu[:BLK], in1=isig[:BLK],
                        op=ALU.mult)
                    y_t = epi.tile([P, D], F32, tag="y_t")
                    nc.scalar.activation(
                        y_t[:BLK], outw[:BLK], AF.Identity, scale=isig[:BLK],
                        bias=nmi[:BLK])
                    if nz["ln_gb"]:
                        nc.vector.tensor_tensor(
                            out=y_t[:BLK], in0=y_t[:BLK], in1=lng_bc[:BLK],
                            op=ALU.mult)
                        nc.vector.tensor_tensor(
                            out=y_t[:BLK], in0=y_t[:BLK], in1=lnb_bc[:BLK],
                            op=ALU.add)
                    e_t = epi.tile([P, D], F32, tag="e_t")
                    nc.scalar.activation(e_t[:BLK], y_t[:BLK], AF.Exp)
                    r_t = epi.tile([P, D], F32, tag="r_t")
                    nc.vector.tensor_scalar(
                        out=r_t[:BLK], in0=y_t[:BLK], scalar1=0.0,
                        scalar2=None, op0=ALU.max)
                    pre = epi.tile([P, D], F32, tag="pre")
                    nc.vector.scalar_tensor_tensor(
                        out=pre[:BLK], in0=e_t[:BLK], scalar=1.0,
                        in1=r_t[:BLK], op0=ALU.min, op1=ALU.add)
                    if l == 0:
                        # x2 - 1 = (x - 2) + (elu + 1)
                        nc.vector.tensor_tensor(
                            out=x2res_sb[:BLK, b, :], in0=pre[:BLK],
                            in1=xres_sb[:BLK, b, :], op=ALU.add)
                        for dt in range(2):
                            tps = etp.tile([P, BLK], F32, space="PSUM",
                                           tag="tps")
                            nc.tensor.transpose(
                                tps[:, :], x2res_sb[0:BLK, b,
                                                    dt * P:(dt + 1) * P],
                                identf_sb[0:BLK, 0:BLK])
                            # +1 restores true x2 for the next-layer GEMM
                            nc.scalar.activation(
                                x2t_sb[:, dt, b * BLK:(b + 1) * BLK],
                                tps[:, :], AF.Copy, bias=1.0)
                        if b % 5 == 4:
                            c0 = (b - 4) * BLK
                            c1 = PPC if b == NBLK - 1 else (b + 1) * BLK
                            nc.sync.dma_start(
                                x2t_loc[:, :, c0:c1].rearrange("d p c -> p d c"),
                                x2t_sb[:, :, c0:c1])
                    else:
                        xout = epi.tile([P, D], F32, tag="xout")
                        nc.vector.tensor_tensor(
                            out=xout[:BLK], in0=pre[:BLK],
                            in1=x2res_sb[:BLK, b, :], op=ALU.add)
                        nc.sync.dma_start(
                            out_x[b * BLK:(b + 1) * BLK, :], xout[:BLK, :])
                    cbase += nchb

        # ================= layer 0 =================
        att0 = cp.tile([P, nchmax * D], BF16, tag="att0")
        nc.sync.dma_start(att0[:], att_rep[0, :, :])
        if nz["b_out"]:
            bout_bc = cp.tile([P, D], F32, tag="bout_bc")
            nc.gpsimd.partition_broadcast(bout_bc[:], b_out[0, :, :])
        if nz["ln_gb"]:
            nc.gpsimd.partition_broadcast(lng_bc[:], ln_gb[0, 0:1, :])
            nc.gpsimd.partition_broadcast(lnb_bc[:], ln_gb[0, 1:2, :])

        with tc.tile_pool(name="xt0", bufs=1) as xp:
            xt_loc = xp.tile([P, 2, NLOC], BF16)
            nc.sync.dma_start(xt_loc[:], x1t_loc[:, :, :])
            xr0 = xr_gemm(0, lambda dt, t:
                          xt_loc[:, dt, t * BLK:(t + 1) * BLK])
            xl_gemm(0, x1t_chunk)
        tc.strict_bb_all_engine_barrier()
        edge_phase(0, xr0, att0)

        tc.strict_bb_all_engine_barrier()
        nc.gpsimd.collective_compute(
            "AllGather", ALU.bypass,
            replica_groups=[list(range(NCORES))],
            ins=[x2t_loc[:, :, :]], outs=[x2t_full[:, :, :, :]])
        # overlap: layer-1 xr GEMM reads local x2^T straight from SBUF
        xr1 = xr_gemm(1, lambda dt, t:
                      x2t_sb[:, dt, t * BLK:(t + 1) * BLK])
        att1 = cp.tile([P, nchmax * D], BF16, tag="att1")
        nc.sync.dma_start(att1[:], att_rep[1, :, :])
        if nz["b_out"]:
            nc.gpsimd.partition_broadcast(bout_bc[:], b_out[1, :, :])
        if nz["ln_gb"]:
            nc.gpsimd.partition_broadcast(lng_bc[:], ln_gb[1, 0:1, :])
            nc.gpsimd.partition_broadcast(lnb_bc[:], ln_gb[1, 1:2, :])
        tc.strict_bb_all_engine_barrier()

        # ================= layer 1 =================
        xl_gemm(1, x2t_chunk)
        tc.strict_bb_all_engine_barrier()
        edge_phase(1, xr1, att1)

    nc.compile()
    return nc


# ---------------------------------------------------------------- interface
_BF = ml_dtypes.bfloat16


def _to_bf16(a):
    return np.asarray(a, np.float32).astype(_BF)


def kernel(x, edge_index, edge_attr, Wl, bl, Wr, br, We, att, bias_out,
           ln_g, ln_b, trace=False):
    x = np.asarray(x, np.float32)
    Wl = np.asarray(Wl, np.float32)
    Wr = np.asarray(Wr, np.float32)
    We = np.asarray(We, np.float32)
    att = np.asarray(att, np.float32)
    bl = np.asarray(bl, np.float32)
    br = np.asarray(br, np.float32)
    bias_out = np.asarray(bias_out, np.float32)
    ln_g = np.asarray(ln_g, np.float32)
    ln_b = np.asarray(ln_b, np.float32)

    nch, totch, per_core = _prep_edges(edge_index, edge_attr)
    nchmax = max(nch)

    nz = {
        "b_lr": bool(np.any(bl) or np.any(br)),
        "b_out": bool(np.any(bias_out)),
        "ln_gb": bool(np.any(ln_g != 1.0) or np.any(ln_b)),
    }
    nc = build_program(nch, totch, nz,
                       sim_safe=(os.environ.get("GAT_SIMSAFE", "0") == "1"))

    # shared (replicated) inputs
    xb = _to_bf16(x)                                   # [N, D]
    x1t_np = np.zeros((P, 2, NCORES, PPC), _BF)
    xv = xb.reshape(NCORES, NLOC, D)
    for k in range(NCORES):
        # x^T columns for core k: node i at column i, i < 2500
        xt_k = xv[k].T.reshape(2, P, NLOC)             # [2, 128, 2500]
        x1t_np[:, :, k, :NLOC] = xt_k.transpose(1, 0, 2)
    we_rep_np = np.zeros((L, 1, NBLK * D), _BF)
    att_rep_np = np.zeros((L, P, nchmax * D), _BF)
    for l in range(L):
        we_rep_np[l, 0] = np.tile(_to_bf16(We[l, 0]), NBLK)
        att_rep_np[l] = np.tile(_to_bf16(att[l].reshape(D)), (P, nchmax))
    iota_np = np.tile(np.arange(P, dtype=np.float32), (P, nchmax)).astype(_BF)
    ident_np = np.eye(P, dtype=np.float32)
    b_lr_np = np.stack([bl, br], axis=1)               # [L, 2, D]
    ln_gb_np = np.stack([ln_g, ln_b], axis=1)          # [L, 2, D]

    shared = {
        "x1t": x1t_np, "w_l": _to_bf16(Wl), "w_r": _to_bf16(Wr),
        "we_rep": we_rep_np, "att_rep": att_rep_np, "iota_big": iota_np,
        "ident_b": ident_np.astype(_BF), "ident_f": ident_np,
        "b_lr": b_lr_np, "b_out": bias_out[:, None, :], "ln_gb": ln_gb_np,
    }
    in_maps = []
    xf = x.reshape(NCORES, NLOC, D)
    for k in range(NCORES):
        m = dict(shared)
        m.update(per_core[k])
        m["x1t_loc"] = np.ascontiguousarray(
            xb[k * NLOC:(k + 1) * NLOC].T.reshape(2, P, NLOC)
            .transpose(1, 0, 2))
        xm2 = np.zeros((P, NBLK, D), np.float32)
        loc = xf[k].reshape(NBLK, BLK, D)
        xm2[:BLK] = (loc - 2.0).transpose(1, 0, 2)
        m["xloc_m2"] = xm2
        in_maps.append(m)

    res = run_bass_kernel_spmd(nc, in_maps, list(range(NCORES)), trace=trace)
    out = np.concatenate([res.results[k]["out_x"] for k in range(NCORES)], 0)
    if trace:
        kernel.last_exec_time_ns = res.exec_time_ns
    return out
